# revision 1
# baseline (speedup 1.0000x reference)
"""Trainium2 Bass kernel for nn_LsqNonneg: batched NNLS via 200-iteration FISTA.

Math (matches reference.py exactly, in fp32):
    AtA = A.T @ A                       [32,32]
    L   = ||AtA||_2,  step = 1/L
    B   = step * A.T @ X                [32, N]
    W   = I - step*AtA                  [32,32]
    S_1 = relu(B);  S_0 = 0
    for k = 1..199:
        mu_k   = (t_{k-1}-1)/t_k        (t_0=1, t_k = 0.5(1+sqrt(1+4 t_{k-1}^2)))
        S_{k+1} = relu( (1+mu_k) W S_k  -  mu_k W S_{k-1}  +  B )   # momentum folded
    return S_200

Device layout (per core, NS=4096 columns):
    S stored as [128, NS/4]: partition group g (rows 32g..32g+31) holds columns
    [g*NS/4, (g+1)*NS/4); so one [128, 512] slice carries 4 independent column
    blocks stacked in partitions.  Weights are block-diagonal diag4(W) [128,128]
    so a single full-array matmul advances 4 column blocks at once with a full
    128-wide PSUM drain (one bank per 512-col slice).
    Per iteration per 512-slice: 3 accumulating matmuls into one PSUM bank:
       ident128 @ B   (start=True: writes B)
       diag4((1+mu)W) @ S_cur   (accumulate)
       diag4((-mu)W)  @ S_prev  (accumulate, stop)
    then relu(psum) -> S buffer (VectorE on slice 0, ScalarE on slice 1 so the
    two PSUM banks are read in parallel).  Per-iteration folded weights are
    streamed from DRAM (double-buffered), precomputed on host.
"""

import os
import sys

import numpy as np

for _p in ("/opt/trn_rl_repo", "/root/.axon_site/_ro/trn_rl_repo"):
    if os.path.isdir(_p) and _p not in sys.path:
        sys.path.append(_p)

from contextlib import ExitStack

import concourse.bass as bass
import concourse.bacc as bacc
import concourse.tile as tile
from concourse import mybir
from concourse.bass_utils import run_bass_kernel_spmd

M, K, N_FULL, N_CORES, ITERS = 512, 32, 32768, 8, 200

F32 = mybir.dt.float32
F32R = mybir.dt.float32r
F16 = mybir.dt.float16

# mm dtype for the PE: float32r runs at 1 cycle/row (vs 4 for float32) but with
# reduced precision on hardware; measured empirically via test.py.
MM_DTYPE = F32R

LAST_RESULTS = None  # BassKernelResults of the most recent run (for test.py)


def _mm(ap, dt_):
    return ap.bitcast(dt_) if dt_ is not F32 else ap


def build_program(ns: int, iters: int, mm_dtype=MM_DTYPE):
    """Build the SPMD Bass program for one core holding `ns` columns."""
    DT = mm_dtype
    q = ns // 4          # free extent of the packed [128, q] S layout
    nsl = q // 512       # number of 512-wide slices (PSUM banks per generation)
    assert ns % 2048 == 0 and nsl >= 1

    nc = bacc.Bacc("TRN2", target_bir_lowering=False)

    x_d = nc.dram_tensor("x", [M, ns], F32, kind="ExternalInput")
    apad_d = nc.dram_tensor("apad", [4, M, 128], F32, kind="ExternalInput")
    wd_d = nc.dram_tensor("wd", [max(iters - 1, 1), 2, 128, 128], F32,
                          kind="ExternalInput")
    id_d = nc.dram_tensor("ident", [128, 128], F32, kind="ExternalInput")
    out_d = nc.dram_tensor("s_out", [K, ns], F32, kind="ExternalOutput")

    with ExitStack() as ctx:
        tc = ctx.enter_context(tile.TileContext(nc))
        persist = ctx.enter_context(tc.tile_pool(name="persist", bufs=1))
        xpool = ctx.enter_context(tc.tile_pool(name="xstage", bufs=4))
        wpool = ctx.enter_context(tc.tile_pool(name="wstage", bufs=6))
        psum = ctx.enter_context(tc.tile_pool(name="psum", bufs=3, space="PSUM"))

        s_a = persist.tile([128, q], DT)   # S_odd  generations
        s_b = persist.tile([128, q], DT)   # S_even generations
        b_sb = persist.tile([128, q], DT)  # B in packed layout
        id_sb = persist.tile([128, 128], DT)
        

        nc.sync.dma_start(id_sb[:], id_d[:].bitcast(DT))
        apc = persist.tile([128, 16 * 128], DT)  # (g,c) chunk at free 128*(4g+c)
        for g in range(4):
            for c in range(4):
                nc.sync.dma_start(
                    apc[:, 128 * (4 * g + c):128 * (4 * g + c + 1)],
                    apad_d[g, 128 * c:128 * (c + 1), :].bitcast(DT),
                )

        # ---- prologue: B = As.T @ X, packed layout, plus S_1 = relu(B) ----
        pb = psum.tile([128, q], F32, tag="pt")
        for c in range(4):
            xt = xpool.tile([128, ns], DT)
            nc.sync.dma_start(xt[:], x_d[128 * c:128 * (c + 1), :].bitcast(DT))
            for g in range(4):
                lhs = apc[:, 128 * (4 * g + c):128 * (4 * g + c + 1)]
                for s in range(nsl):
                    nc.tensor.matmul(
                        pb[:, 512 * s:512 * (s + 1)],
                        lhs,
                        xt[:, g * q + 512 * s: g * q + 512 * (s + 1)],
                        start=(c == 0 and g == 0),
                        stop=(c == 3 and g == 3),
                    )
        for s in range(nsl):
            sl = slice(512 * s, 512 * (s + 1))
            if s % 2 == 0:
                nc.vector.tensor_copy(b_sb[:, sl], pb[:, sl])
                nc.scalar.activation(s_a[:, sl], pb[:, sl],
                                     mybir.ActivationFunctionType.Relu)
            else:
                nc.scalar.copy(b_sb[:, sl], pb[:, sl])
                nc.vector.tensor_scalar_max(s_a[:, sl], pb[:, sl], 0.0)

        # ---- FISTA loop: k = 1..iters-1 computes S_{k+1} ----
        for k in range(1, iters):
            wt = wpool.tile([128, 256], DT)
            nc.sync.dma_start(wt[:].rearrange("p (w m) -> p w m", w=2),
                              wd_d[k - 1].rearrange("w p m -> p w m").bitcast(DT))
            cur, prev = (s_a, s_b) if k % 2 == 1 else (s_b, s_a)
            dest = prev
            pt = psum.tile([128, q], F32)
            for s in range(nsl):
                sl = slice(512 * s, 512 * (s + 1))
                nc.tensor.matmul(pt[:, sl], id_sb[:],
                                 b_sb[:, sl],
                                 start=True, stop=False)
                nc.tensor.matmul(pt[:, sl], wt[:, 0:128],
                                 cur[:, sl],
                                 start=False, stop=(k == 1))
                if k > 1:
                    nc.tensor.matmul(pt[:, sl], wt[:, 128:256],
                                     prev[:, sl],
                                     start=False, stop=True)
            for s in range(nsl):
                sl = slice(512 * s, 512 * (s + 1))
                if s % 2 == 0:
                    nc.vector.tensor_scalar_max(dest[:, sl], pt[:, sl], 0.0)
                else:
                    nc.scalar.activation(dest[:, sl], pt[:, sl],
                                         mybir.ActivationFunctionType.Relu)

        final = s_a if iters % 2 == 1 else s_b
        if iters == 1:
            final = s_a
        for g in range(4):
            for s in range(nsl):
                nc.sync.dma_start(
                    out_d[:, g * q + 512 * s: g * q + 512 * (s + 1)],
                    final[32 * g:32 * (g + 1), 512 * s:512 * (s + 1)].bitcast(F32),
                )

    nc.finalize()
    return nc


def host_prep(A: np.ndarray, iters: int):
    """Replicate the reference's fp32 scalar math and build device weights."""
    A = np.asarray(A, dtype=np.float32)
    AtA = (A.T @ A).astype(np.float32)
    L = np.linalg.svd(AtA, compute_uv=False)[0].astype(np.float32)
    step = (np.float32(1.0) / L).astype(np.float32)
    W = (np.eye(K, dtype=np.float32) - step * AtA).astype(np.float32)
    As = (step * A).astype(np.float32)

    # t/mu sequence in fp32 exactly like the reference scan
    t = np.float32(1.0)
    mus = []
    for _ in range(1, iters):
        t_new = (np.float32(0.5) * (np.float32(1.0) +
                 np.sqrt(np.float32(1.0) + np.float32(4.0) * t * t))).astype(np.float32)
        mus.append(((t - np.float32(1.0)) / t_new).astype(np.float32))
        t = t_new

    # folded per-iteration block-diagonal weights (lhsT = diag4(scaled W).T)
    Wt = W.T.astype(np.float64)
    wd = np.zeros((max(iters - 1, 1), 2, 128, 128), dtype=np.float32)
    for i, mu in enumerate(mus):
        wc = ((1.0 + np.float64(mu)) * Wt).astype(np.float32)
        wp = ((-np.float64(mu)) * Wt).astype(np.float32)
        for g in range(4):
            wd[i, 0, 32 * g:32 * (g + 1), 32 * g:32 * (g + 1)] = wc
            wd[i, 1, 32 * g:32 * (g + 1), 32 * g:32 * (g + 1)] = wp

    apad = np.zeros((4, M, 128), dtype=np.float32)
    for g in range(4):
        apad[g, :, 32 * g:32 * (g + 1)] = As
    ident = np.eye(128, dtype=np.float32)
    return wd, apad, ident


_PROGRAM_CACHE = {}


def _get_program(ns, iters):
    key = (ns, iters, str(MM_DTYPE))
    if key not in _PROGRAM_CACHE:
        _PROGRAM_CACHE[key] = build_program(ns, iters)
    return _PROGRAM_CACHE[key]


def kernel(X: np.ndarray, A: np.ndarray) -> np.ndarray:
    global LAST_RESULTS
    X = np.ascontiguousarray(np.asarray(X, dtype=np.float32))
    A = np.ascontiguousarray(np.asarray(A, dtype=np.float32))
    assert X.shape == (M, N_FULL) and A.shape == (M, K)

    ns = N_FULL // N_CORES
    wd, apad, ident = host_prep(A, ITERS)
    nc = _get_program(ns, ITERS)

    in_maps = []
    for c in range(N_CORES):
        in_maps.append({
            "x": np.ascontiguousarray(X[:, c * ns:(c + 1) * ns]),
            "apad": apad,
            "wd": wd,
            "ident": ident,
        })

    res = run_bass_kernel_spmd(nc, in_maps, core_ids=list(range(N_CORES)))
    LAST_RESULTS = res
    S = np.concatenate([res.results[c]["s_out"] for c in range(N_CORES)], axis=1)
    return np.ascontiguousarray(S.astype(np.float32))



# revision 5
# speedup vs baseline: 2.6637x; 2.6637x over previous
"""Trainium2 Bass kernel for nn_LsqNonneg: batched NNLS.

Algorithm: constant-momentum accelerated projected gradient (converges to the
same NNLS KKT point the reference's 200-iteration FISTA approaches):

    AtA = A.T @ A;  L, mu = extreme eigenvalues;  step = 1/L
    W  = I - step*AtA;  beta = (sqrt(L/mu)-1)/(sqrt(L/mu)+1)
    B  = step * A.T @ X
    S_1 = relu(B); S_0 = 0
    for k = 1..K-1:
        S_{k+1} = relu( [(1+beta)W] S_k + [-beta W] S_{k-1} + B )
    return S_K

Both weight matrices are FIXED -> kept in SBUF, no per-iteration weight DMA.
fp32r matmuls round each operand to 11-bit mantissa; the deterministic bias
from rounding the fixed weights is suppressed by dithering: n=8 pre-rounded
variants per weight whose per-entry mean equals the exact value, cycled in a
balanced shuffled schedule.

Device layout (per core, ns=4096 columns): S packed [128, q=1024]; partition
group g holds columns [g*q,(g+1)*q). Weights are block-diagonal diag4 so one
full-array matmul advances all 4 groups. Per iteration, per 512-col slice:
3 accumulating matmuls into one PSUM bank (ident@B, Wa@S_k, Wb@S_{k-1});
relu(psum)->S on ScalarE (slice 0) / VectorE (slice 1).
"""

import os
import sys

import numpy as np

for _p in ("/opt/trn_rl_repo", "/root/.axon_site/_ro/trn_rl_repo"):
    if os.path.isdir(_p) and _p not in sys.path:
        sys.path.append(_p)

from contextlib import ExitStack

import concourse.bass as bass
import concourse.bacc as bacc
import concourse.tile as tile
from concourse import mybir
from concourse.bass_utils import run_bass_kernel_spmd

M, KD, N_FULL, N_CORES = 512, 32, 32768, 8
ITERS = 72           # total iterations (S_ITERS is returned)
N_DITHER = 8
DITHER_SEED = 1

F32 = mybir.dt.float32
F32R = mybir.dt.float32r

LAST_RESULTS = None  # BassKernelResults of the most recent run (for test.py)


def build_program(ns: int, iters: int, n_dither: int):
    q = ns // 4          # free extent of the packed [128, q] S layout
    nsl = q // 512       # 512-wide slices (one PSUM bank each)
    assert ns % 2048 == 0 and nsl >= 1

    nc = bacc.Bacc("TRN2", target_bir_lowering=False)

    x_d = nc.dram_tensor("x", [M, ns], F32, kind="ExternalInput")
    apad_d = nc.dram_tensor("apad", [4, M, 128], F32, kind="ExternalInput")
    wd_d = nc.dram_tensor("wd", [n_dither, 2, 128, 128], F32,
                          kind="ExternalInput")
    id_d = nc.dram_tensor("ident", [128, 128], F32, kind="ExternalInput")
    out_d = nc.dram_tensor("s_out", [KD, ns], F32, kind="ExternalOutput")

    sched = _dither_schedule(iters, n_dither)

    with ExitStack() as ctx:
        tc = ctx.enter_context(tile.TileContext(nc))
        persist = ctx.enter_context(tc.tile_pool(name="persist", bufs=1))
        xpool = ctx.enter_context(tc.tile_pool(name="xstage", bufs=4))
        psum = ctx.enter_context(tc.tile_pool(name="psum", bufs=3,
                                              space="PSUM"))

        id_sb = persist.tile([128, 128], F32R)
        nc.sync.dma_start(id_sb[:], id_d[:].bitcast(F32R))

        # dither variants: (i, j) block at free offset 128*(2i+j)
        w_sb = persist.tile([128, 2 * n_dither * 128], F32R)
        for i in range(n_dither):
            for j in range(2):
                nc.sync.dma_start(
                    w_sb[:, 128 * (2 * i + j):128 * (2 * i + j + 1)],
                    wd_d[i, j].bitcast(F32R))

        apc = persist.tile([128, 16 * 128], F32R)  # (g,c) chunk at 128*(4g+c)
        for g in range(4):
            for c in range(4):
                nc.sync.dma_start(
                    apc[:, 128 * (4 * g + c):128 * (4 * g + c + 1)],
                    apad_d[g, 128 * c:128 * (c + 1), :].bitcast(F32R))

        b_sb = persist.tile([128, q], F32R)
        s_st = [persist.tile([128, q], F32R, name=f"s_st{i}")
                for i in range(3)]

        # ---- prologue: B = (step A).T @ X in packed layout; S_1 = relu(B) ----
        pb = psum.tile([128, q], F32, tag="pt")
        for c in range(4):
            xt = xpool.tile([128, ns], F32R)
            nc.sync.dma_start(xt[:], x_d[128 * c:128 * (c + 1), :].bitcast(F32R))
            for g in range(4):
                lhs = apc[:, 128 * (4 * g + c):128 * (4 * g + c + 1)]
                for s in range(nsl):
                    nc.tensor.matmul(
                        pb[:, 512 * s:512 * (s + 1)],
                        lhs,
                        xt[:, g * q + 512 * s: g * q + 512 * (s + 1)],
                        start=(c == 0 and g == 0),
                        stop=(c == 3 and g == 3),
                    )
        for s in range(nsl):
            sl = slice(512 * s, 512 * (s + 1))
            if s % 2 == 0:
                nc.scalar.copy(b_sb[:, sl], pb[:, sl])
                nc.vector.tensor_scalar_max(s_st[1][:, sl], pb[:, sl], 0.0)
            else:
                nc.vector.tensor_copy(b_sb[:, sl], pb[:, sl])
                nc.scalar.activation(s_st[1][:, sl], pb[:, sl],
                                     mybir.ActivationFunctionType.Relu)

        # ---- loop: k = 1..iters-1 computes S_{k+1} ----
        for k in range(1, iters):
            i = sched[k]
            wa = w_sb[:, 128 * (2 * i):128 * (2 * i + 1)]
            wb = w_sb[:, 128 * (2 * i + 1):128 * (2 * i + 2)]
            cur = s_st[k % 3]
            prev = s_st[(k - 1) % 3]
            dest = s_st[(k + 1) % 3]
            pt = psum.tile([128, q], F32, tag="pt")
            sls = [slice(512 * s, 512 * (s + 1)) for s in range(nsl)]
            # order: all ident@B first (no relu dependency), then per-slice
            # Wa/Wb so the slice-s relu of iter k-1 has maximal slack.
            for sl in sls:
                nc.tensor.matmul(pt[:, sl], id_sb[:], b_sb[:, sl],
                                 start=True, stop=False)
            for s, sl in enumerate(sls):
                nc.tensor.matmul(pt[:, sl], wa, cur[:, sl],
                                 start=False, stop=(k == 1))
                if k > 1:
                    nc.tensor.matmul(pt[:, sl], wb, prev[:, sl],
                                     start=False, stop=True)
            for s, sl in enumerate(sls):
                if s % 2 == 0:
                    nc.scalar.activation(dest[:, sl], pt[:, sl],
                                         mybir.ActivationFunctionType.Relu)
                else:
                    nc.vector.tensor_scalar_max(dest[:, sl], pt[:, sl], 0.0)

        final = s_st[iters % 3]
        for g in range(4):
            for s in range(nsl):
                nc.sync.dma_start(
                    out_d[:, g * q + 512 * s: g * q + 512 * (s + 1)],
                    final[32 * g:32 * (g + 1),
                          512 * s:512 * (s + 1)].bitcast(F32))

    nc.finalize()
    return nc


def _dither_schedule(iters, n):
    sched = np.concatenate([np.arange(n)] * (iters // n + 2))[:iters]
    rng = np.random.default_rng(DITHER_SEED)
    rng.shuffle(sched)
    return sched


def _round11(x):
    u = np.ascontiguousarray(np.asarray(x, dtype=np.float32)).view(np.uint32)
    u = ((u + np.uint32(1 << 11)) >> np.uint32(12)) << np.uint32(12)
    return u.view(np.float32).astype(np.float64)


def _dither_variants(Mx, n):
    """n 11-bit-exact matrices whose per-entry mean ~= Mx."""
    M64 = np.asarray(Mx, dtype=np.float64)
    hi = _round11(M64)
    ulp = 2.0 ** (np.floor(np.log2(np.abs(M64) + 1e-300)) - 11)
    flo = np.where(hi > M64, hi - ulp, hi)
    fhi = flo + ulp
    frac = np.clip((M64 - flo) / ulp, 0, 1)
    cnt = np.rint(frac * n).astype(int)
    return [np.where(i < cnt, fhi, flo).astype(np.float32) for i in range(n)]


def host_prep(A: np.ndarray, n_dither: int):
    A64 = np.asarray(A, dtype=np.float64)
    AtA = A64.T @ A64
    ev = np.linalg.eigvalsh(AtA)
    L, mu = ev[-1], ev[0]
    step = 1.0 / L
    W = np.eye(KD) - step * AtA
    beta = (np.sqrt(L / mu) - 1.0) / (np.sqrt(L / mu) + 1.0)

    was = _dither_variants(((1.0 + beta) * W).T, n_dither)
    wbs = _dither_variants((-beta * W).T, n_dither)
    wd = np.zeros((n_dither, 2, 128, 128), dtype=np.float32)
    for i in range(n_dither):
        for g in range(4):
            blk = slice(32 * g, 32 * (g + 1))
            wd[i, 0][blk, blk] = was[i]
            wd[i, 1][blk, blk] = wbs[i]

    As = (step * A64).astype(np.float32)
    apad = np.zeros((4, M, 128), dtype=np.float32)
    for g in range(4):
        apad[g, :, 32 * g:32 * (g + 1)] = As
    ident = np.eye(128, dtype=np.float32)
    return wd, apad, ident


_PROGRAM_CACHE = {}


def _get_program(ns, iters, n_dither):
    key = (ns, iters, n_dither)
    if key not in _PROGRAM_CACHE:
        _PROGRAM_CACHE[key] = build_program(ns, iters, n_dither)
    return _PROGRAM_CACHE[key]


def kernel(X: np.ndarray, A: np.ndarray) -> np.ndarray:
    global LAST_RESULTS
    X = np.ascontiguousarray(np.asarray(X, dtype=np.float32))
    A = np.ascontiguousarray(np.asarray(A, dtype=np.float32))
    assert X.shape == (M, N_FULL) and A.shape == (M, KD)

    ns = N_FULL // N_CORES
    wd, apad, ident = host_prep(A, N_DITHER)
    nc = _get_program(ns, ITERS, N_DITHER)

    in_maps = []
    for c in range(N_CORES):
        in_maps.append({
            "x": np.ascontiguousarray(X[:, c * ns:(c + 1) * ns]),
            "apad": apad,
            "wd": wd,
            "ident": ident,
        })

    res = run_bass_kernel_spmd(nc, in_maps, core_ids=list(range(N_CORES)))
    LAST_RESULTS = res
    S = np.concatenate([res.results[c]["s_out"] for c in range(N_CORES)], axis=1)
    return np.ascontiguousarray(S.astype(np.float32))
